# revision 17
# baseline (speedup 1.0000x reference)
"""Sliding-window attention probe kernel for 8 TRN2 NeuronCores.

Problem (hardcoded): B=2, S=2048, D=2048, H=16, DP=128, WINDOW=512.
  q/k/v = x @ w{q,k,v} + b{q,k,v}   (per-head dim 128)
  scores = q k^T / sqrt(128), sliding causal window 512, softmax
  o = attn @ v ; out = o @ wo + bo[0]          -> (B, S)

Key algebraic simplification: wo is a vector, so
  out[t] = sum_h (sum_k attn_h[t,k] * vw_h[k]) + bo,  vw_h = x @ (wv_h @ wo_h) + bv_h.wo_h
V is never materialized. Each core handles 2 heads (tensor-parallel over heads),
computes per-head numerator/denominator of the softmax-weighted vw sums, and the
host divides + sums over heads.

Precision design (validated against the f32 oracle on the fixed seed):
- All 16-bit storage is fp16 (e5m10), not bf16: 8x less rounding noise at
  identical PE/DVE throughput. fp16-everything alone measures 2.9e-4 rel err.
- The q/k projections run in fp8 e4m3 with DoubleRow perf mode: one matmul
  contracts TWO 128-deep k-subtiles (2 fp8 weights per PE cell), halving the
  dominant projection cost. Host-side power-of-2 scales keep tensors inside
  TRN e4m3's +-240 range; the inverse scale folds into the existing
  PSUM->SBUF bias-add (tensor_scalar mult+add, same op count).
  Measured rel err fp16+QK-fp8: 1.69e-2 (gate 2e-2). The wvo projection must
  stay fp16: quantizing vw to fp8 alone costs 1.5e-2.
"""

import numpy as np
import ml_dtypes

import concourse.bass as bass
import concourse.tile as tile
from concourse import bacc, mybir
from concourse.bass_utils import run_bass_kernel_spmd

F16 = mybir.dt.float16
F8 = mybir.dt.float8e4
F32 = mybir.dt.float32
B, S, D = 2, 2048, 2048
H, DP = 16, 128
WINDOW = 512
N_CORES = 8
HPC = H // N_CORES  # heads per core = 2
T = B * S  # 4096 tokens
NQT = S // 128  # 16 query tiles per batch
NTB = S // 512  # 4 512-token blocks per batch
NDT = D // 128  # 16 contraction tiles
NDP = NDT // 2  # 8 DoubleRow dt-pairs

Q_FP8 = True  # False -> q projection stays fp16 (more error margin)

_CACHE = {}


def _build_program(cq, ck):
    nc = bacc.Bacc(
        "TRN2", target_bir_lowering=False, debug=False, num_devices=N_CORES
    )

    # block-major: [128, B*NTB, NDT, 512] so one (batch, tblock) chunk is a
    # long per-partition-contiguous run (fewer, bigger DMA descriptors)
    x8 = nc.dram_tensor("x8", [128, B * NTB, NDT, 512], F8, kind="ExternalInput").ap()
    xt = nc.dram_tensor("xt", [128, B * NTB, NDT, 512], F16, kind="ExternalInput").ap()
    # head-major: [:, h] is one contiguous 2KB-per-partition run
    wq8 = nc.dram_tensor("wq8", [128, HPC, NDT, 128], F8, kind="ExternalInput").ap()
    wk8 = nc.dram_tensor("wk8", [128, HPC, NDT, 128], F8, kind="ExternalInput").ap()
    wqh = nc.dram_tensor("wqh", [128, HPC, NDT, 128], F16, kind="ExternalInput").ap()
    wvo = nc.dram_tensor("wvo", [128, NDT, HPC], F16, kind="ExternalInput").ap()
    bq = nc.dram_tensor("bq", [128, HPC], F32, kind="ExternalInput").ap()
    bk = nc.dram_tensor("bk", [128, HPC], F32, kind="ExternalInput").ap()
    bvo = nc.dram_tensor("bvo", [HPC, 1], F32, kind="ExternalInput").ap()
    mdiag = nc.dram_tensor("mdiag", [128, 128], F16, kind="ExternalInput").ap()
    mfar = nc.dram_tensor("mfar", [128, 128], F16, kind="ExternalInput").ap()
    ident = nc.dram_tensor("ident", [128, 128], F16, kind="ExternalInput").ap()
    nd = nc.dram_tensor("nd", [B, HPC, 2, S], F32, kind="ExternalOutput").ap()

    with tile.TileContext(nc) as tc:
        _body(tc, x8, xt, wq8, wk8, wqh, wvo, bq, bk, bvo, mdiag, mfar, ident, nd, cq, ck)

    nc.compile()
    return nc


def _body(tc, x8, xt, wq8, wk8, wqh, wvo, bq, bk, bvo, mdiag, mfar, ident, nd, cq, ck):
    nc = tc.nc
    with (
        tc.tile_pool(name="consts", bufs=1) as consts,
        tc.tile_pool(name="x8pool", bufs=3) as x8pool,
        tc.tile_pool(name="xpool", bufs=3) as xpool,
        tc.tile_pool(name="qkpool", bufs=8) as qkpool,
        tc.tile_pool(name="vwpool", bufs=2) as vwpool,
        tc.tile_pool(name="vw1pool", bufs=24) as vw1pool,
        tc.tile_pool(name="exppool", bufs=26) as exppool,
        tc.tile_pool(name="outpool", bufs=2) as outpool,
        tc.tile_pool(name="pj", bufs=2, space="PSUM") as pj,
        tc.tile_pool(name="sc", bufs=2, space="PSUM") as sc,
        tc.tile_pool(name="sm", bufs=2, space="PSUM") as sm,
    ):
        # PE warmup: dummy matmuls with no input deps keep the PE array busy
        # from kernel start so the HAM clock ramps while the first DMAs land.
        # The PE queue is in-order, so too many would delay real work.
        warm = consts.tile([128, 128], F16)
        nc.vector.memset(warm[:], 1.0)
        warm_ps = pj.tile([128, 512], F32, tag="pj", name="warm_ps")
        for _ in range(96):
            nc.tensor.matmul(
                warm_ps[:, 0:128], lhsT=warm[:], rhs=warm[:],
                start=True, stop=True, skip_group_check=True,
            )

        # One DGE queue sustains only ~120-150 GB/s, so every phase spreads
        # across the sync/scalar/gpsimd queues. Phases are ordered by
        # time-of-need: x8[b0] (~12us) -> xt[b0] (~38us) -> x8[b1] (~55us)
        # -> xt[b1] (~95us). Queues are FIFO, so later phases never delay
        # earlier ones.
        queues = [nc.sync, nc.scalar, nc.gpsimd]
        qi = [0]

        def dma(out, in_):
            queues[qi[0] % 3].dma_start(out=out, in_=in_)
            qi[0] += 1

        x8_tiles = {
            (b, tb): x8pool.tile([128, NDT, 512], F8, tag="x8s", name="x8s", bufs=5)
            for b in range(B)
            for tb in range(NTB)
        }
        xt_tiles = {
            (b, tb): xpool.tile([128, NDT, 512], F16, tag="xs", name="xs", bufs=4)
            for b in range(B)
            for tb in range(NTB)
        }

        def load_x8(b):
            for tb in range(NTB):
                blk = b * NTB + tb
                xs = x8_tiles[(b, tb)]
                for c in range(2):
                    dma(xs[:, 8 * c : 8 * c + 8, :], x8[:, blk, 8 * c : 8 * c + 8, :])

        def load_xt(b):
            for tb in range(NTB):
                blk = b * NTB + tb
                xs = xt_tiles[(b, tb)]
                for c in range(2):
                    dma(xs[:, 8 * c : 8 * c + 8, :], xt[:, blk, 8 * c : 8 * c + 8, :])

        # All weights/constants first — they are tiny (KBs) but gate the
        # bias-adds and masks; queued behind MBs of x they arrive 20+ us late.
        wq8_sb = consts.tile([128, HPC, NDT, 128], F8)
        nc.scalar.dma_start(out=wq8_sb[:, 0], in_=wq8[:, 0])
        wk8_sb = consts.tile([128, HPC, NDT, 128], F8)
        nc.gpsimd.dma_start(out=wk8_sb[:, 0], in_=wk8[:, 0])
        bq_sb = consts.tile([128, HPC], F32)
        nc.scalar.dma_start(out=bq_sb[:], in_=bq[:])
        bk_sb = consts.tile([128, HPC], F32)
        nc.gpsimd.dma_start(out=bk_sb[:], in_=bk[:])
        nc.scalar.dma_start(out=wq8_sb[:, 1], in_=wq8[:, 1])
        nc.gpsimd.dma_start(out=wk8_sb[:, 1], in_=wk8[:, 1])
        if not Q_FP8:
            wqh_sb = consts.tile([128, HPC, NDT, 128], F16)
            nc.scalar.dma_start(out=wqh_sb[:, 0], in_=wqh[:, 0])
            nc.scalar.dma_start(out=wqh_sb[:, 1], in_=wqh[:, 1])
        mdiag_sb = consts.tile([128, 128], F16)
        nc.scalar.dma_start(out=mdiag_sb[:], in_=mdiag[:])
        mfar_sb = consts.tile([128, 128], F16)
        nc.scalar.dma_start(out=mfar_sb[:], in_=mfar[:])
        ident_sb = consts.tile([128, 128], F16)
        nc.scalar.dma_start(out=ident_sb[:], in_=ident[:])
        wvo_sb = consts.tile([128, NDT, HPC], F16)
        nc.gpsimd.dma_start(out=wvo_sb[:], in_=wvo[:])
        bvo_sb = consts.tile([HPC, 1], F32)
        nc.gpsimd.dma_start(out=bvo_sb[:], in_=bvo[:])

        # bulk phases, each striped over all three queues
        load_x8(0)
        load_xt(0)
        load_x8(1)
        load_xt(1)

        def emit_proj(w_sb, b_sb, out_t, out_col0, xs_list, width):
            """fp16 projection: accumulate NDT k-tiles for each tblock."""
            for tb in range(NTB):
                ps = pj.tile([width, 512], F32, tag="pj", name="pj")
                for dt in range(NDT):
                    nc.tensor.matmul(
                        ps[:],
                        lhsT=w_sb[:, dt, out_col0 : out_col0 + width],
                        rhs=xs_list[tb][:, dt, :],
                        start=(dt == 0),
                        stop=(dt == NDT - 1),
                    )
                nc.vector.tensor_scalar_add(
                    out_t[:, tb * 512 : tb * 512 + 512], ps[:], b_sb
                )

        def emit_proj_pair(h, qk_t, x8_list, xt_list, wqh_sb=None):
            """q/k projections for head h: fp8 DoubleRow, 8 dt-pair matmuls.

            The PSUM->SBUF move applies the inverse quantization scale and
            the bias in one tensor_scalar op.
            """
            for tb in range(NTB):
                specs = [
                    (wk8_sb, bk_sb, ck, 1),
                    (wq8_sb, bq_sb, cq, 0) if Q_FP8 else (wqh_sb, bq_sb, 1.0, 0),
                ]
                for w_sb, b_sb, c, which in specs:
                    ps = pj.tile([128, 512], F32, tag="pj", name="pj")
                    if w_sb.dtype == F8:
                        for p in range(NDP):
                            nc.tensor.matmul(
                                ps[:],
                                lhsT=w_sb[:, h, 2 * p : 2 * p + 2, :],
                                rhs=x8_list[tb][:, 2 * p : 2 * p + 2, :],
                                start=(p == 0),
                                stop=(p == NDP - 1),
                                perf_mode=mybir.MatmulPerfMode.DoubleRow,
                            )
                    else:
                        for dt in range(NDT):
                            nc.tensor.matmul(
                                ps[:],
                                lhsT=w_sb[:, h, dt, :],
                                rhs=xt_list[tb][:, dt, :],
                                start=(dt == 0),
                                stop=(dt == NDT - 1),
                            )
                    nc.vector.tensor_scalar(
                        qk_t[h][which][:, tb * 512 : tb * 512 + 512],
                        ps[:],
                        float(c),
                        b_sb[:, h : h + 1],
                        mybir.AluOpType.mult,
                        mybir.AluOpType.add,
                    )

        def emit_scores(qT, kT, js=None, exps=None):
            """Windowed scores + exp + masks for the given key tiles."""
            if exps is None:
                exps = []
            for j in js if js is not None else range(NQT):
                wd = min(j + 4, NQT - 1) - j + 1  # query tiles in span
                w = 128 * wd
                w1 = min(w, 512)
                e = exppool.tile([128, 640], F16, tag="expT", name="expT")
                s1 = sc.tile([128, 640], F32, tag="sc", name="sc")
                nc.tensor.matmul(
                    s1[:, 0:w1],
                    lhsT=kT[:, 128 * j : 128 * j + 128],
                    rhs=qT[:, 128 * j : 128 * j + w1],
                    start=True,
                    stop=True,
                )
                if w > 512:
                    nc.tensor.matmul(
                        s1[:, 512:640],
                        lhsT=kT[:, 128 * j : 128 * j + 128],
                        rhs=qT[:, 128 * j + 512 : 128 * j + 640],
                        start=True,
                        stop=True,
                        skip_group_check=True,
                    )
                nc.scalar.activation(
                    e[:, 0:w], s1[:, 0:w], mybir.ActivationFunctionType.Exp
                )
                # causal mask on the diagonal block (query tile i == j)
                nc.vector.tensor_mul(e[:, 0:128], e[:, 0:128], mdiag_sb[:])
                # window-cut mask on the far block (query tile i == j+4)
                if wd == 5:
                    nc.vector.tensor_mul(e[:, 512:640], e[:, 512:640], mfar_sb[:])
                exps.append(e)
            return exps

        def emit_numdem_bank(h, m, exps, vw1s, ob):
            """512-wide bank-aligned numerator/denominator accumulation.

            Bank m covers queries [512m, 512m+512) and gets contributions
            from key tiles j in [4m-4, 4m+3]. Key tile j = 4m covers the
            whole bank, so it goes first with start=True; every other piece
            then accumulates into a fully-written region (keeps per-element
            has_written uniform for each matmul, as the simulator requires).
            """
            blk = sm.tile([2, 512], F32, tag="sm", name="ndps")
            js = [4 * m] + [
                j
                for j in range(max(0, 4 * m - 4), min(4 * m + 4, NQT))
                if j != 4 * m
            ]
            for idx, j in enumerate(js):
                s0 = 128 * j
                s1 = min(s0 + 640, S)
                a = max(s0, 512 * m)
                bnd = min(s1, 512 * m + 512)
                nc.tensor.matmul(
                    blk[:, a - 512 * m : bnd - 512 * m],
                    lhsT=vw1s[j][:, h, :],
                    rhs=exps[j][:, a - s0 : bnd - s0],
                    start=(idx == 0),
                    stop=(idx == len(js) - 1),
                    skip_group_check=True,
                )
            nc.vector.tensor_copy(ob[:, 512 * m : 512 * m + 512], blk[:])

        for b in range(B):
            x8_list = [x8_tiles[(b, tb)] for tb in range(NTB)]
            xt_list = [xt_tiles[(b, tb)] for tb in range(NTB)]
            qk_t = [
                [qkpool.tile([128, S], F16, tag="qkT", name="qkT") for _ in range(2)]
                for _ in range(HPC)
            ]
            vw_sb = vwpool.tile([HPC, S], F16, tag="vw")

            # head 0 projections, then its scores/exp — the exp overlaps the
            # head 1 projections on the otherwise-idle scalar engine.
            emit_proj_pair(0, qk_t, x8_list, xt_list,
                           None if Q_FP8 else wqh_sb)
            exps0 = emit_scores(qk_t[0][0], qk_t[0][1])

            emit_proj_pair(1, qk_t, x8_list, xt_list,
                           None if Q_FP8 else wqh_sb)
            emit_proj(wvo_sb, bvo_sb[:], vw_sb, 0, xt_list, HPC)

            # vw transpose: [HPC, S] -> per key tile [128, HPC] with a ones
            # column appended for the denominator row
            vw1s = []
            for j in range(NQT):
                vt = sm.tile([128, HPC], F16, tag="sm", name="vt")
                nc.tensor.transpose(
                    vt[:],
                    vw_sb[:, 128 * j : 128 * j + 128],
                    ident_sb[0:HPC, 0:HPC],
                )
                vw1 = vw1pool.tile([128, HPC, 2], F16, tag="vw1", name="vw1")
                nc.vector.memset(vw1[:], 1.0)
                nc.vector.tensor_copy(vw1[:, :, 0], vt[:])
                vw1s.append(vw1)

            # interleave head 1 scores with head 0 numdem so the head 1
            # exp work streams on ACT while the PE drains head 0
            ob0 = outpool.tile([2, S], F32, tag="ob", name="ob")
            ob1 = outpool.tile([2, S], F32, tag="ob", name="ob")
            exps1 = []
            for m in range(S // 512):
                emit_scores(
                    qk_t[1][0], qk_t[1][1], js=range(4 * m, 4 * m + 4), exps=exps1
                )
                emit_numdem_bank(0, m, exps0, vw1s, ob0)
                nc.sync.dma_start(
                    out=nd[b, 0][:, 512 * m : 512 * m + 512],
                    in_=ob0[:, 512 * m : 512 * m + 512],
                )
            for m in range(S // 512):
                emit_numdem_bank(1, m, exps1, vw1s, ob1)
                nc.sync.dma_start(
                    out=nd[b, 1][:, 512 * m : 512 * m + 512],
                    in_=ob1[:, 512 * m : 512 * m + 512],
                )


def _pow2_scale(a, max_abs=240.0):
    return np.float32(2.0 ** np.floor(np.log2(max_abs / np.abs(a).max())))


def _prep_inputs(x, wq_f, bq_f, wk_f, bk_f, wv_f, bv_f, wo_f, bo_f):
    """Host-side shard + layout prep. Returns (in_maps, bo_scalar, cq, ck)."""
    fh = np.float16
    f8 = ml_dtypes.float8_e4m3
    rsq = np.float32(1.0 / np.sqrt(np.float32(DP)))

    xt_full = np.ascontiguousarray(x.reshape(T, D).T)  # [D, T] f32
    # block-major [128, B*NTB, NDT, 512]:
    # element (p, blk, dt, c) = xT[dt*128+p, blk*512+c]
    xt_t = np.ascontiguousarray(
        xt_full.reshape(NDT, 128, B * NTB, 512).transpose(1, 2, 0, 3)
    )
    xt_tiled = xt_t.astype(fh)
    sx = _pow2_scale(x)
    x8_tiled = (xt_t * sx).astype(f8)

    wq_pre = wq_f * rsq  # fold 1/sqrt(DP)
    swq = _pow2_scale(wq_pre)
    swk = _pow2_scale(wk_f)
    cq = np.float32(1.0) / (sx * swq)
    ck = np.float32(1.0) / (sx * swk)

    mdiag_np = (
        np.arange(128)[None, :] >= np.arange(128)[:, None]
    ).astype(fh)  # keep a >= b  [b(part), a(free)]
    mfar_np = (
        np.arange(128)[:, None] > np.arange(128)[None, :]
    ).astype(fh)  # keep b > a
    ident_np = np.eye(128, dtype=fh)

    in_maps = []
    for c in range(N_CORES):
        c0 = c * HPC * DP
        c1 = c0 + HPC * DP
        wq_c = wq_pre[:, c0:c1].astype(np.float32)
        bq_c = (bq_f[c0:c1] * rsq).astype(np.float32)
        wk_c = wk_f[:, c0:c1].astype(np.float32)
        bk_c = bk_f[c0:c1].astype(np.float32)

        # fold wv @ wo per head -> wvo [D, HPC]
        wvo_c = np.empty((D, HPC), np.float64)
        bvo_c = np.empty((HPC, 1), np.float32)
        for h in range(HPC):
            g0 = c0 + h * DP
            g1 = g0 + DP
            wvo_c[:, h] = wv_f[:, g0:g1].astype(np.float64) @ wo_f[g0:g1].astype(
                np.float64
            )
            bvo_c[h, 0] = np.float32(
                bv_f[g0:g1].astype(np.float64) @ wo_f[g0:g1].astype(np.float64)
            )
        wvo_c = wvo_c.astype(np.float32)

        def tile_w(w, dt, scale=None):  # [D, C] -> [128, NDT, C]
            t = np.ascontiguousarray(w.reshape(NDT, 128, -1).transpose(1, 0, 2))
            if scale is not None:
                t = t * scale
            return t.astype(dt)

        def tile_w_hm(w, dt, scale=None):  # [D, HPC*128] -> [128, HPC, NDT, 128]
            t = w.reshape(NDT, 128, HPC, 128).transpose(1, 2, 0, 3)
            t = np.ascontiguousarray(t)
            if scale is not None:
                t = t * scale
            return t.astype(dt)

        in_maps.append(
            {
                "x8": x8_tiled,
                "xt": xt_tiled,
                "wq8": tile_w_hm(wq_c, f8, swq),
                "wk8": tile_w_hm(wk_c, f8, swk),
                "wqh": tile_w_hm(wq_c, fh),
                "wvo": tile_w(wvo_c, fh),
                "bq": np.ascontiguousarray(bq_c.reshape(HPC, 128).T),
                "bk": np.ascontiguousarray(bk_c.reshape(HPC, 128).T),
                "bvo": bvo_c,
                "mdiag": mdiag_np,
                "mfar": mfar_np,
                "ident": ident_np,
            }
        )
    return in_maps, np.float32(bo_f[0]), cq, ck


def kernel(x, wq, bq, wk, bk, wv, bv, wo, bo):
    x = np.asarray(x, np.float32)
    in_maps, bo_s, cq, ck = _prep_inputs(
        x,
        np.asarray(wq, np.float32),
        np.asarray(bq, np.float32),
        np.asarray(wk, np.float32),
        np.asarray(bk, np.float32),
        np.asarray(wv, np.float32),
        np.asarray(bv, np.float32),
        np.asarray(wo, np.float32),
        np.asarray(bo, np.float32),
    )

    key = (float(cq), float(ck))
    if _CACHE.get("key") != key:
        _CACHE["nc"] = _build_program(cq, ck)
        _CACHE["key"] = key
    nc = _CACHE["nc"]

    res = run_bass_kernel_spmd(nc, in_maps, core_ids=list(range(N_CORES)))
    out = np.zeros((B, S), np.float32)
    for c in range(N_CORES):
        nd = res.results[c]["nd"]  # [B, HPC, 2, S]
        out += (nd[:, :, 0, :] / nd[:, :, 1, :]).sum(axis=1)
    return out + bo_s


# revision 21
# speedup vs baseline: 1.1232x; 1.1232x over previous
"""Sliding-window attention probe kernel for 8 TRN2 NeuronCores.

Problem (hardcoded): B=2, S=2048, D=2048, H=16, DP=128, WINDOW=512.
  q/k/v = x @ w{q,k,v} + b{q,k,v}   (per-head dim 128)
  scores = q k^T / sqrt(128), sliding causal window 512, softmax
  o = attn @ v ; out = o @ wo + bo[0]          -> (B, S)

Key algebraic simplification: wo is a vector, so
  out[t] = sum_h (sum_k attn_h[t,k] * vw_h[k]) + bo,  vw_h = x @ (wv_h @ wo_h) + bv_h.wo_h
V is never materialized. Each core handles 2 heads (tensor-parallel over heads),
computes per-head numerator/denominator of the softmax-weighted vw sums, and the
host divides + sums over heads.

Precision design (validated against the f32 oracle on the fixed seed):
- All 16-bit storage is fp16 (e5m10), not bf16: 8x less rounding noise at
  identical PE/DVE throughput. fp16-everything alone measures 2.9e-4 rel err.
- The q/k projections run in fp8 e4m3 with DoubleRow perf mode: one matmul
  contracts TWO 128-deep k-subtiles (2 fp8 weights per PE cell), halving the
  dominant projection cost. Host-side power-of-2 scales keep tensors inside
  TRN e4m3's +-240 range; the inverse scale folds into the existing
  PSUM->SBUF bias-add (tensor_scalar mult+add, same op count).
  Measured rel err fp16+QK-fp8: 1.69e-2 (gate 2e-2). The wvo projection must
  stay fp16: quantizing vw to fp8 alone costs 1.5e-2.
"""

import numpy as np
import ml_dtypes

import concourse.bass as bass
import concourse.tile as tile
from concourse import bacc, mybir
from concourse.bass_utils import run_bass_kernel_spmd

F16 = mybir.dt.float16
F8 = mybir.dt.float8e4
F32 = mybir.dt.float32
B, S, D = 2, 2048, 2048
H, DP = 16, 128
WINDOW = 512
N_CORES = 8
HPC = H // N_CORES  # heads per core = 2
T = B * S  # 4096 tokens
NQT = S // 128  # 16 query tiles per batch
NTB = S // 512  # 4 512-token blocks per batch
NDT = D // 128  # 16 contraction tiles
NDP = NDT // 2  # 8 DoubleRow dt-pairs

Q_FP8 = True  # False -> q projection stays fp16 (more error margin)

_CACHE = {}


def _build_program(cq, ck):
    nc = bacc.Bacc(
        "TRN2", target_bir_lowering=False, debug=False, num_devices=N_CORES
    )

    # block-major: [128, B*NTB, NDT, 512] so one (batch, tblock) chunk is a
    # long per-partition-contiguous run (fewer, bigger DMA descriptors)
    x8 = nc.dram_tensor("x8", [128, B * NTB, NDT, 512], F8, kind="ExternalInput").ap()
    xt = nc.dram_tensor("xt", [128, B * NTB, NDT, 512], F16, kind="ExternalInput").ap()
    # head-major: [:, h] is one contiguous 2KB-per-partition run
    wq8 = nc.dram_tensor("wq8", [128, HPC, NDT, 128], F8, kind="ExternalInput").ap()
    wk8 = nc.dram_tensor("wk8", [128, HPC, NDT, 128], F8, kind="ExternalInput").ap()
    wqh = nc.dram_tensor("wqh", [128, HPC, NDT, 128], F16, kind="ExternalInput").ap()
    wvo = nc.dram_tensor("wvo", [128, NDT, HPC], F16, kind="ExternalInput").ap()
    bqk = nc.dram_tensor("bqk", [128, 2, HPC], F32, kind="ExternalInput").ap()
    bvo = nc.dram_tensor("bvo", [HPC, 1], F32, kind="ExternalInput").ap()
    # packed masks: [:,0]=mdiag, [:,1]=mfar, [:,2]=ident — one DMA trigger
    masks = nc.dram_tensor("masks", [128, 3, 128], F16, kind="ExternalInput").ap()
    nd = nc.dram_tensor("nd", [B, HPC, 2, S], F32, kind="ExternalOutput").ap()

    with tile.TileContext(nc) as tc:
        _body(tc, x8, xt, wq8, wk8, wqh, wvo, bqk, bvo, masks, nd, cq, ck)

    nc.compile()
    return nc


def _body(tc, x8, xt, wq8, wk8, wqh, wvo, bqk, bvo, masks, nd, cq, ck):
    nc = tc.nc
    with (
        tc.tile_pool(name="consts", bufs=1) as consts,
        tc.tile_pool(name="x8pool", bufs=3) as x8pool,
        tc.tile_pool(name="xpool", bufs=3) as xpool,
        tc.tile_pool(name="qkpool", bufs=8) as qkpool,
        tc.tile_pool(name="vwpool", bufs=2) as vwpool,
        tc.tile_pool(name="vw1pool", bufs=24) as vw1pool,
        tc.tile_pool(name="exppool", bufs=26) as exppool,
        tc.tile_pool(name="outpool", bufs=2) as outpool,
        tc.tile_pool(name="pj", bufs=2, space="PSUM") as pj,
        tc.tile_pool(name="sc", bufs=2, space="PSUM") as sc,
        tc.tile_pool(name="sm", bufs=2, space="PSUM") as sm,
    ):
        # PE warmup: dummy matmuls with no input deps keep the PE array busy
        # from kernel start so the HAM clock ramps while the first DMAs land.
        # The PE queue is in-order, so too many would delay real work.
        warm = consts.tile([128, 128], F16)
        nc.vector.memset(warm[:], 1.0)
        warm_ps = pj.tile([128, 512], F32, tag="pj", name="warm_ps")
        for _ in range(96):
            nc.tensor.matmul(
                warm_ps[:, 0:128], lhsT=warm[:], rhs=warm[:],
                start=True, stop=True, skip_group_check=True,
            )

        # One DGE queue sustains only ~120-150 GB/s, so every phase spreads
        # across the sync/scalar/gpsimd queues. Phases are ordered by
        # time-of-need: x8[b0] (~12us) -> xt[b0] (~38us) -> x8[b1] (~55us)
        # -> xt[b1] (~95us). Queues are FIFO, so later phases never delay
        # earlier ones.
        queues = [nc.sync, nc.scalar, nc.gpsimd]
        qi = [0]

        def dma(out, in_):
            queues[qi[0] % 3].dma_start(out=out, in_=in_)
            qi[0] += 1

        x8_tiles = {
            (b, tb): x8pool.tile([128, NDT, 512], F8, tag="x8s", name="x8s", bufs=5)
            for b in range(B)
            for tb in range(NTB)
        }
        xt_tiles = {
            (b, tb): xpool.tile([128, NDT, 512], F16, tag="xs", name="xs", bufs=4)
            for b in range(B)
            for tb in range(NTB)
        }

        def load_x8(b):
            for tb in range(NTB):
                blk = b * NTB + tb
                xs = x8_tiles[(b, tb)]
                for c in range(2):
                    dma(xs[:, 8 * c : 8 * c + 8, :], x8[:, blk, 8 * c : 8 * c + 8, :])

        def load_xt(b):
            for tb in range(NTB):
                blk = b * NTB + tb
                xs = xt_tiles[(b, tb)]
                for c in range(2):
                    dma(xs[:, 8 * c : 8 * c + 8, :], xt[:, blk, 8 * c : 8 * c + 8, :])

        # Trigger order is everything: each dma_start costs ~0.6-3.5us of
        # sequencer time, so the critical x8[b0] stream goes right behind
        # the head-0 weights + biases; all other constants ride mid-stream
        # as two packed transfers.
        wq8_sb = consts.tile([128, HPC, NDT, 128], F8)
        nc.scalar.dma_start(out=wq8_sb[:, 0], in_=wq8[:, 0])
        wk8_sb = consts.tile([128, HPC, NDT, 128], F8)
        nc.gpsimd.dma_start(out=wk8_sb[:, 0], in_=wk8[:, 0])
        bqk_sb = consts.tile([128, 2, HPC], F32)
        nc.sync.dma_start(out=bqk_sb[:], in_=bqk[:])
        bq_sb = bqk_sb[:, 0]
        bk_sb = bqk_sb[:, 1]

        load_x8(0)

        masks_sb = consts.tile([128, 3, 128], F16)
        nc.sync.dma_start(out=masks_sb[:], in_=masks[:])
        mdiag_sb = masks_sb[:, 0]
        mfar_sb = masks_sb[:, 1]
        ident_sb = masks_sb[:, 2]
        nc.scalar.dma_start(out=wq8_sb[:, 1], in_=wq8[:, 1])
        nc.gpsimd.dma_start(out=wk8_sb[:, 1], in_=wk8[:, 1])
        if not Q_FP8:
            wqh_sb = consts.tile([128, HPC, NDT, 128], F16)
            nc.scalar.dma_start(out=wqh_sb[:, 0], in_=wqh[:, 0])
            nc.scalar.dma_start(out=wqh_sb[:, 1], in_=wqh[:, 1])
        wvo_sb = consts.tile([128, NDT, HPC], F16)
        nc.gpsimd.dma_start(out=wvo_sb[:], in_=wvo[:])
        bvo_sb = consts.tile([HPC, 1], F32)
        nc.gpsimd.dma_start(out=bvo_sb[:], in_=bvo[:])

        # remaining bulk phases, each striped over all three queues
        load_xt(0)
        load_x8(1)
        load_xt(1)

        def emit_proj(w_sb, b_sb, out_t, out_col0, xs_list, width):
            """fp16 projection: accumulate NDT k-tiles for each tblock."""
            for tb in range(NTB):
                ps = pj.tile([width, 512], F32, tag="pj", name="pj")
                for dt in range(NDT):
                    nc.tensor.matmul(
                        ps[:],
                        lhsT=w_sb[:, dt, out_col0 : out_col0 + width],
                        rhs=xs_list[tb][:, dt, :],
                        start=(dt == 0),
                        stop=(dt == NDT - 1),
                    )
                nc.vector.tensor_scalar_add(
                    out_t[:, tb * 512 : tb * 512 + 512], ps[:], b_sb
                )

        def emit_proj_pair(h, qk_t, x8_list, xt_list, wqh_sb=None):
            """q/k projections for head h: fp8 DoubleRow, 8 dt-pair matmuls.

            The PSUM->SBUF move applies the inverse quantization scale and
            the bias in one tensor_scalar op.
            """
            for tb in range(NTB):
                specs = [
                    (wk8_sb, bk_sb, ck, 1),
                    (wq8_sb, bq_sb, cq, 0) if Q_FP8 else (wqh_sb, bq_sb, 1.0, 0),
                ]
                for w_sb, b_sb, c, which in specs:
                    ps = pj.tile([128, 512], F32, tag="pj", name="pj")
                    if w_sb.dtype == F8:
                        for p in range(NDP):
                            nc.tensor.matmul(
                                ps[:],
                                lhsT=w_sb[:, h, 2 * p : 2 * p + 2, :],
                                rhs=x8_list[tb][:, 2 * p : 2 * p + 2, :],
                                start=(p == 0),
                                stop=(p == NDP - 1),
                                perf_mode=mybir.MatmulPerfMode.DoubleRow,
                            )
                    else:
                        for dt in range(NDT):
                            nc.tensor.matmul(
                                ps[:],
                                lhsT=w_sb[:, h, dt, :],
                                rhs=xt_list[tb][:, dt, :],
                                start=(dt == 0),
                                stop=(dt == NDT - 1),
                            )
                    nc.vector.tensor_scalar(
                        qk_t[h][which][:, tb * 512 : tb * 512 + 512],
                        ps[:],
                        float(c),
                        b_sb[:, h : h + 1],
                        mybir.AluOpType.mult,
                        mybir.AluOpType.add,
                    )

        def emit_scores(qT, kT, js=None, exps=None):
            """Windowed scores + exp + masks for the given key tiles."""
            if exps is None:
                exps = []
            for j in js if js is not None else range(NQT):
                wd = min(j + 4, NQT - 1) - j + 1  # query tiles in span
                w = 128 * wd
                w1 = min(w, 512)
                e = exppool.tile([128, 640], F16, tag="expT", name="expT")
                s1 = sc.tile([128, 640], F32, tag="sc", name="sc")
                nc.tensor.matmul(
                    s1[:, 0:w1],
                    lhsT=kT[:, 128 * j : 128 * j + 128],
                    rhs=qT[:, 128 * j : 128 * j + w1],
                    start=True,
                    stop=True,
                )
                if w > 512:
                    nc.tensor.matmul(
                        s1[:, 512:640],
                        lhsT=kT[:, 128 * j : 128 * j + 128],
                        rhs=qT[:, 128 * j + 512 : 128 * j + 640],
                        start=True,
                        stop=True,
                        skip_group_check=True,
                    )
                nc.scalar.activation(
                    e[:, 0:w], s1[:, 0:w], mybir.ActivationFunctionType.Exp
                )
                # causal mask on the diagonal block (query tile i == j)
                nc.vector.tensor_mul(e[:, 0:128], e[:, 0:128], mdiag_sb[:])
                # window-cut mask on the far block (query tile i == j+4)
                if wd == 5:
                    nc.vector.tensor_mul(e[:, 512:640], e[:, 512:640], mfar_sb[:])
                exps.append(e)
            return exps

        def emit_numdem_bank(h, m, exps, vw1s, ob):
            """512-wide bank-aligned numerator/denominator accumulation.

            Bank m covers queries [512m, 512m+512) and gets contributions
            from key tiles j in [4m-4, 4m+3]. Key tile j = 4m covers the
            whole bank, so it goes first with start=True; every other piece
            then accumulates into a fully-written region (keeps per-element
            has_written uniform for each matmul, as the simulator requires).
            """
            blk = sm.tile([2, 512], F32, tag="sm", name="ndps")
            js = [4 * m] + [
                j
                for j in range(max(0, 4 * m - 4), min(4 * m + 4, NQT))
                if j != 4 * m
            ]
            for idx, j in enumerate(js):
                s0 = 128 * j
                s1 = min(s0 + 640, S)
                a = max(s0, 512 * m)
                bnd = min(s1, 512 * m + 512)
                nc.tensor.matmul(
                    blk[:, a - 512 * m : bnd - 512 * m],
                    lhsT=vw1s[j][:, h, :],
                    rhs=exps[j][:, a - s0 : bnd - s0],
                    start=(idx == 0),
                    stop=(idx == len(js) - 1),
                    skip_group_check=True,
                )
            nc.vector.tensor_copy(ob[:, 512 * m : 512 * m + 512], blk[:])

        for b in range(B):
            x8_list = [x8_tiles[(b, tb)] for tb in range(NTB)]
            xt_list = [xt_tiles[(b, tb)] for tb in range(NTB)]
            qk_t = [
                [qkpool.tile([128, S], F16, tag="qkT", name="qkT") for _ in range(2)]
                for _ in range(HPC)
            ]
            vw_sb = vwpool.tile([HPC, S], F16, tag="vw")

            # head 0 projections, then its scores/exp — the exp overlaps the
            # head 1 projections on the otherwise-idle scalar engine.
            emit_proj_pair(0, qk_t, x8_list, xt_list,
                           None if Q_FP8 else wqh_sb)
            exps0 = emit_scores(qk_t[0][0], qk_t[0][1])

            emit_proj_pair(1, qk_t, x8_list, xt_list,
                           None if Q_FP8 else wqh_sb)
            emit_proj(wvo_sb, bvo_sb[:], vw_sb, 0, xt_list, HPC)

            # vw transpose: [HPC, S] -> per key tile [128, HPC] with a ones
            # column appended for the denominator row
            vw1s = []
            for j in range(NQT):
                vt = sm.tile([128, HPC], F16, tag="sm", name="vt")
                nc.tensor.transpose(
                    vt[:],
                    vw_sb[:, 128 * j : 128 * j + 128],
                    ident_sb[0:HPC, 0:HPC],
                )
                vw1 = vw1pool.tile([128, HPC, 2], F16, tag="vw1", name="vw1")
                nc.vector.memset(vw1[:], 1.0)
                nc.vector.tensor_copy(vw1[:, :, 0], vt[:])
                vw1s.append(vw1)

            # interleave head 1 scores with head 0 numdem so the head 1
            # exp work streams on ACT while the PE drains head 0
            ob0 = outpool.tile([2, S], F32, tag="ob", name="ob")
            ob1 = outpool.tile([2, S], F32, tag="ob", name="ob")
            exps1 = []
            for m in range(S // 512):
                emit_scores(
                    qk_t[1][0], qk_t[1][1], js=range(4 * m, 4 * m + 4), exps=exps1
                )
                emit_numdem_bank(0, m, exps0, vw1s, ob0)
                nc.sync.dma_start(
                    out=nd[b, 0][:, 512 * m : 512 * m + 512],
                    in_=ob0[:, 512 * m : 512 * m + 512],
                )
            for m in range(S // 512):
                emit_numdem_bank(1, m, exps1, vw1s, ob1)
                nc.sync.dma_start(
                    out=nd[b, 1][:, 512 * m : 512 * m + 512],
                    in_=ob1[:, 512 * m : 512 * m + 512],
                )


def _pow2_scale(a, max_abs=240.0):
    return np.float32(2.0 ** np.floor(np.log2(max_abs / np.abs(a).max())))


def _prep_inputs(x, wq_f, bq_f, wk_f, bk_f, wv_f, bv_f, wo_f, bo_f):
    """Host-side shard + layout prep. Returns (in_maps, bo_scalar, cq, ck)."""
    fh = np.float16
    f8 = ml_dtypes.float8_e4m3
    rsq = np.float32(1.0 / np.sqrt(np.float32(DP)))

    xt_full = np.ascontiguousarray(x.reshape(T, D).T)  # [D, T] f32
    # block-major [128, B*NTB, NDT, 512]:
    # element (p, blk, dt, c) = xT[dt*128+p, blk*512+c]
    xt_t = np.ascontiguousarray(
        xt_full.reshape(NDT, 128, B * NTB, 512).transpose(1, 2, 0, 3)
    )
    xt_tiled = xt_t.astype(fh)
    sx = _pow2_scale(x)
    x8_tiled = (xt_t * sx).astype(f8)

    wq_pre = wq_f * rsq  # fold 1/sqrt(DP)
    swq = _pow2_scale(wq_pre)
    swk = _pow2_scale(wk_f)
    cq = np.float32(1.0) / (sx * swq)
    ck = np.float32(1.0) / (sx * swk)

    mdiag_np = (
        np.arange(128)[None, :] >= np.arange(128)[:, None]
    ).astype(fh)  # keep a >= b  [b(part), a(free)]
    mfar_np = (
        np.arange(128)[:, None] > np.arange(128)[None, :]
    ).astype(fh)  # keep b > a
    ident_np = np.eye(128, dtype=fh)

    in_maps = []
    for c in range(N_CORES):
        c0 = c * HPC * DP
        c1 = c0 + HPC * DP
        wq_c = wq_pre[:, c0:c1].astype(np.float32)
        bq_c = (bq_f[c0:c1] * rsq).astype(np.float32)
        wk_c = wk_f[:, c0:c1].astype(np.float32)
        bk_c = bk_f[c0:c1].astype(np.float32)

        # fold wv @ wo per head -> wvo [D, HPC]
        wvo_c = np.empty((D, HPC), np.float64)
        bvo_c = np.empty((HPC, 1), np.float32)
        for h in range(HPC):
            g0 = c0 + h * DP
            g1 = g0 + DP
            wvo_c[:, h] = wv_f[:, g0:g1].astype(np.float64) @ wo_f[g0:g1].astype(
                np.float64
            )
            bvo_c[h, 0] = np.float32(
                bv_f[g0:g1].astype(np.float64) @ wo_f[g0:g1].astype(np.float64)
            )
        wvo_c = wvo_c.astype(np.float32)

        def tile_w(w, dt, scale=None):  # [D, C] -> [128, NDT, C]
            t = np.ascontiguousarray(w.reshape(NDT, 128, -1).transpose(1, 0, 2))
            if scale is not None:
                t = t * scale
            return t.astype(dt)

        def tile_w_hm(w, dt, scale=None):  # [D, HPC*128] -> [128, HPC, NDT, 128]
            t = w.reshape(NDT, 128, HPC, 128).transpose(1, 2, 0, 3)
            t = np.ascontiguousarray(t)
            if scale is not None:
                t = t * scale
            return t.astype(dt)

        in_maps.append(
            {
                "x8": x8_tiled,
                "xt": xt_tiled,
                "wq8": tile_w_hm(wq_c, f8, swq),
                "wk8": tile_w_hm(wk_c, f8, swk),
                "wqh": tile_w_hm(wq_c, fh),
                "wvo": tile_w(wvo_c, fh),
                "bqk": np.ascontiguousarray(
                    np.stack(
                        [bq_c.reshape(HPC, 128).T, bk_c.reshape(HPC, 128).T], axis=1
                    )
                ),
                "bvo": bvo_c,
                "masks": np.ascontiguousarray(
                    np.stack([mdiag_np, mfar_np, ident_np], axis=1)
                ),
            }
        )
    return in_maps, np.float32(bo_f[0]), cq, ck


def kernel(x, wq, bq, wk, bk, wv, bv, wo, bo):
    x = np.asarray(x, np.float32)
    in_maps, bo_s, cq, ck = _prep_inputs(
        x,
        np.asarray(wq, np.float32),
        np.asarray(bq, np.float32),
        np.asarray(wk, np.float32),
        np.asarray(bk, np.float32),
        np.asarray(wv, np.float32),
        np.asarray(bv, np.float32),
        np.asarray(wo, np.float32),
        np.asarray(bo, np.float32),
    )

    key = (float(cq), float(ck))
    if _CACHE.get("key") != key:
        _CACHE["nc"] = _build_program(cq, ck)
        _CACHE["key"] = key
    nc = _CACHE["nc"]

    res = run_bass_kernel_spmd(nc, in_maps, core_ids=list(range(N_CORES)))
    out = np.zeros((B, S), np.float32)
    for c in range(N_CORES):
        nd = res.results[c]["nd"]  # [B, HPC, 2, S]
        out += (nd[:, :, 0, :] / nd[:, :, 1, :]).sum(axis=1)
    return out + bo_s
